# revision 14
# baseline (speedup 1.0000x reference)
"""Trainium2 Bass kernel for int64 quantized linear (nn_Linear_int_90950227460261).

Computes acc = x @ W^T (int, exact), q = acc // 4096, r = acc % 4096.

Math: |x|,|w| <= 128 so both are exact in bf16, and every fp32 partial sum of
the accumulation stays far below 2^24 for these inputs, so a bf16 matmul with
fp32 PSUM accumulation reproduces the integer accumulator bit-exactly. The
div/mod is done on-chip in int32 (arith shift right 12 / and 4095).

Sharding: tensor-parallel over out_features. Each of the 8 cores gets the full
x (transposed to [K, M] bf16) and a [K, 512] slice of W^T, computes
acc_c = W_c x^T as [512, M], and writes q/r int16 slices. Host concatenates
along the feature axis and transposes back.
"""

import numpy as np
import ml_dtypes

import concourse.mybir as mybir
import concourse.tile as tile
from concourse import bacc, bass_utils

B, S, K = 2, 2048, 4096
M = B * S                 # 4096 tokens
N_TOT = 4096              # out features
N_CORES = 8
N_CORE = N_TOT // N_CORES  # 512 features per core
P = 128
KB = K // P               # 32 k-blocks
M_TILE = 512
M_TILES = M // M_TILE     # 8
N_BLKS = N_CORE // P      # 4

BF16 = ml_dtypes.bfloat16

_NC = None
LAST_RESULTS = None  # BassKernelResults of the most recent run (for profiling)


def _build_program():
    nc = bacc.Bacc("TRN2", target_bir_lowering=False, debug=False)
    # x ships as int8 (4x less HBM traffic), upconverted to bf16 by the
    # otherwise-idle GpSimd engine; w ships as bf16 directly (only 4 MB).
    xt_d = nc.dram_tensor("xt", [K, M], mybir.dt.int8, kind="ExternalInput").ap()
    wt_d = nc.dram_tensor("wt", [K, N_CORE], mybir.dt.bfloat16, kind="ExternalInput").ap()
    # q and r packed as (q << 16) | r per element; host unpacks.
    qr_d = nc.dram_tensor("qr", [N_CORE, M], mybir.dt.int32, kind="ExternalOutput").ap()

    with tile.TileContext(nc) as tc:
        with (
            tc.tile_pool(name="w", bufs=1) as wpool,
            tc.tile_pool(name="x", bufs=3) as xpool,
            tc.tile_pool(name="acc", bufs=4) as apool,
            tc.tile_pool(name="out", bufs=4) as opool,
            tc.tile_pool(name="psum", bufs=8, space="PSUM") as ppool,
        ):
            # Per-k-block weight tiles (resident whole kernel): precise deps
            # so the first matmul only waits on its own 128 KB chunk. The wt/x
            # chunk DMAs for the first slab are interleaved below so the
            # (wt[0], x[0]) pair lands first and PE starts ~15 us earlier.
            wt_sb = [None] * KB

            def load_w(kb):
                w_t = wpool.tile([P, N_CORE], mybir.dt.bfloat16, tag=f"wt{kb}",
                                 name=f"wt{kb}")
                nc.sync.dma_start(w_t[:], wt_d[kb * P:(kb + 1) * P, :])
                wt_sb[kb] = w_t

            def load_x(mt, kb):
                x8_t = xpool.tile([P, M_TILE], mybir.dt.int8, tag=f"x8_{kb}",
                                  name=f"x8_{mt}_{kb}")
                nc.sync.dma_start(
                    x8_t[:],
                    xt_d[kb * P:(kb + 1) * P, mt * M_TILE:(mt + 1) * M_TILE],
                )
                x_t = xpool.tile([P, M_TILE], mybir.dt.bfloat16, tag=f"x{kb}",
                                 name=f"x{mt}_{kb}")
                nc.gpsimd.tensor_copy(x_t[:], x8_t[:])
                return x_t

            for mt in range(M_TILES):
                xt_sb = []
                for kb in range(KB):
                    if mt == 0:
                        load_w(kb)
                    xt_sb.append(load_x(mt, kb))

                ps = []
                for nb in range(N_BLKS):
                    ps_t = ppool.tile([P, M_TILE], mybir.dt.float32, tag="ps", name=f"ps{mt}_{nb}")
                    ps.append(ps_t)
                # kb-outer, nb-inner: PE chunk consumption (~4 MMs / 256 KB)
                # matches DMA arrival rate, so startup doesn't stall.
                for kb in range(KB):
                    for nb in range(N_BLKS):
                        nc.tensor.matmul(
                            ps[nb][:],
                            wt_sb[kb][:, nb * P:(nb + 1) * P],
                            xt_sb[kb][:],
                            start=(kb == 0),
                            stop=(kb == KB - 1),
                        )
                for nb in range(N_BLKS):
                    acc = apool.tile([P, M_TILE], mybir.dt.int32, tag="acc")
                    nc.scalar.copy(acc[:], ps[nb][:])
                    q_t = opool.tile([P, M_TILE], mybir.dt.int32, tag="q")
                    r_t = opool.tile([P, M_TILE], mybir.dt.int32, tag="r")
                    nc.vector.tensor_scalar(
                        q_t[:], acc[:], 12, 16,
                        mybir.AluOpType.arith_shift_right,
                        mybir.AluOpType.arith_shift_left,
                    )
                    nc.vector.tensor_scalar(
                        r_t[:], acc[:], 4095, None, mybir.AluOpType.bitwise_and
                    )
                    nc.vector.tensor_tensor(
                        q_t[:], q_t[:], r_t[:], mybir.AluOpType.bitwise_or
                    )
                    # ACT's HWDGE ring: outputs don't queue behind input DMAs
                    nc.scalar.dma_start(
                        qr_d[nb * P:(nb + 1) * P, mt * M_TILE:(mt + 1) * M_TILE],
                        q_t[:],
                    )
    nc.compile()
    return nc


def _get_nc():
    global _NC
    if _NC is None:
        _NC = _build_program()
    return _NC


def kernel(x: np.ndarray, weight: np.ndarray):
    global LAST_RESULTS
    x = np.asarray(x)
    weight = np.asarray(weight)
    assert x.shape == (B, S, K) and weight.shape == (N_TOT, K)

    # Host prep: transpose + cast to bf16 (exact: |v| <= 128)
    xt = np.ascontiguousarray(x.reshape(M, K).T.astype(np.int8))  # [K, M]
    in_maps = []
    for c in range(N_CORES):
        wt_c = weight[c * N_CORE:(c + 1) * N_CORE, :].T.astype(np.float32).astype(BF16)
        in_maps.append({"xt": xt, "wt": np.ascontiguousarray(wt_c)})

    nc = _get_nc()
    res = bass_utils.run_bass_kernel_spmd(nc, in_maps, core_ids=list(range(N_CORES)))
    LAST_RESULTS = res

    qr = np.concatenate([r_["qr"] for r_ in res.results], axis=0)  # [N_TOT, M] int32
    qr = np.ascontiguousarray(qr.T)  # [M, N_TOT]
    q = (qr >> 16).astype(np.int64).reshape(B, S, N_TOT)
    r = (qr & 0xFFFF).astype(np.int64).reshape(B, S, N_TOT)
    return (q, r)


# revision 15
# speedup vs baseline: 2.0389x; 2.0389x over previous
"""Trainium2 Bass kernel for int64 quantized linear (nn_Linear_int_90950227460261).

Computes acc = x @ W^T (int, exact), q = acc // 4096, r = acc % 4096.

Math: |x|,|w| <= 128 so both are exact in bf16, and every fp32 partial sum of
the accumulation stays far below 2^24 for these inputs, so a bf16 matmul with
fp32 PSUM accumulation reproduces the integer accumulator bit-exactly. The
div/mod is done on-chip in int32 (arith shift right 12 / and 4095).

Sharding: tensor-parallel over out_features. Each of the 8 cores gets the full
x (transposed to [K, M] bf16) and a [K, 512] slice of W^T, computes
acc_c = W_c x^T as [512, M], and writes q/r int16 slices. Host concatenates
along the feature axis and transposes back.
"""

import numpy as np
import ml_dtypes

import concourse.mybir as mybir
import concourse.tile as tile
from concourse import bacc, bass_utils

B, S, K = 2, 2048, 4096
M = B * S                 # 4096 tokens
N_TOT = 4096              # out features
N_CORES = 8
N_CORE = N_TOT // N_CORES  # 512 features per core
P = 128
KB = K // P               # 32 k-blocks
M_TILE = 512
M_TILES = M // M_TILE     # 8
N_BLKS = N_CORE // P      # 4

BF16 = ml_dtypes.bfloat16

# k-blocks shipped as bf16 via HWDGE vs int8 via casting SWDGE
BF16_KBS = [kb for kb in range(KB) if kb % 4 == 0]
I8_KBS = [kb for kb in range(KB) if kb % 4 != 0]
BF16_KB_IDX = {kb: j for j, kb in enumerate(BF16_KBS)}
I8_KB_IDX = {kb: j for j, kb in enumerate(I8_KBS)}

_NC = None
LAST_RESULTS = None  # BassKernelResults of the most recent run (for profiling)


def _build_program():
    nc = bacc.Bacc("TRN2", target_bir_lowering=False, debug=False)
    # Hybrid input path: every 4th k-block of x (and all of w) ships as bf16
    # over the fast HWDGE ring; the rest of x ships as int8 and is cast to
    # bf16 inline by the SWDGE DMA datapath (~72 GB/s aggregate). Keeps both
    # DMA streams below the PE consumption rate with no staging tiles.
    xtb_d = nc.dram_tensor("xtb", [len(BF16_KBS) * P, M], mybir.dt.bfloat16,
                           kind="ExternalInput").ap()
    xt8_d = nc.dram_tensor("xt8", [len(I8_KBS) * P, M], mybir.dt.int8,
                           kind="ExternalInput").ap()
    wt_d = nc.dram_tensor("wt", [K, N_CORE], mybir.dt.bfloat16, kind="ExternalInput").ap()
    # q and r packed as (q << 16) | r per element; host unpacks.
    qr_d = nc.dram_tensor("qr", [N_CORE, M], mybir.dt.int32, kind="ExternalOutput").ap()

    with tile.TileContext(nc) as tc:
        with (
            tc.tile_pool(name="w", bufs=1) as wpool,
            tc.tile_pool(name="x", bufs=3) as xpool,
            tc.tile_pool(name="acc", bufs=4) as apool,
            tc.tile_pool(name="out", bufs=4) as opool,
            tc.tile_pool(name="psum", bufs=8, space="PSUM") as ppool,
        ):
            # Per-k-block weight tiles (resident whole kernel): precise deps
            # so the first matmul only waits on its own 128 KB chunk. The wt/x
            # chunk DMAs for the first slab are interleaved below so the
            # (wt[0], x[0]) pair lands first and PE starts ~15 us earlier.
            wt_sb = [None] * KB

            def load_w(kb):
                w_t = wpool.tile([P, N_CORE], mybir.dt.bfloat16, tag=f"wt{kb}",
                                 name=f"wt{kb}")
                nc.sync.dma_start(w_t[:], wt_d[kb * P:(kb + 1) * P, :])
                wt_sb[kb] = w_t

            def load_x(mt, kb):
                x_t = xpool.tile([P, M_TILE], mybir.dt.bfloat16, tag=f"x{kb}",
                                 name=f"x{mt}_{kb}")
                mc = slice(mt * M_TILE, (mt + 1) * M_TILE)
                if kb in BF16_KB_IDX:
                    j = BF16_KB_IDX[kb]
                    nc.sync.dma_start(x_t[:], xtb_d[j * P:(j + 1) * P, mc])
                else:
                    j = I8_KB_IDX[kb]
                    nc.gpsimd.dma_start(x_t[:], xt8_d[j * P:(j + 1) * P, mc])
                return x_t

            for mt in range(M_TILES):
                xt_sb = []
                for kb in range(KB):
                    if mt == 0:
                        load_w(kb)
                    xt_sb.append(load_x(mt, kb))

                ps = []
                for nb in range(N_BLKS):
                    ps_t = ppool.tile([P, M_TILE], mybir.dt.float32, tag="ps", name=f"ps{mt}_{nb}")
                    ps.append(ps_t)
                # kb-outer, nb-inner: PE chunk consumption (~4 MMs / 256 KB)
                # matches DMA arrival rate, so startup doesn't stall.
                for kb in range(KB):
                    for nb in range(N_BLKS):
                        nc.tensor.matmul(
                            ps[nb][:],
                            wt_sb[kb][:, nb * P:(nb + 1) * P],
                            xt_sb[kb][:],
                            start=(kb == 0),
                            stop=(kb == KB - 1),
                        )
                for nb in range(N_BLKS):
                    acc = apool.tile([P, M_TILE], mybir.dt.int32, tag="acc")
                    nc.scalar.copy(acc[:], ps[nb][:])
                    q_t = opool.tile([P, M_TILE], mybir.dt.int32, tag="q")
                    r_t = opool.tile([P, M_TILE], mybir.dt.int32, tag="r")
                    nc.vector.tensor_scalar(
                        q_t[:], acc[:], 12, 16,
                        mybir.AluOpType.arith_shift_right,
                        mybir.AluOpType.arith_shift_left,
                    )
                    nc.vector.tensor_scalar(
                        r_t[:], acc[:], 4095, None, mybir.AluOpType.bitwise_and
                    )
                    nc.vector.tensor_tensor(
                        q_t[:], q_t[:], r_t[:], mybir.AluOpType.bitwise_or
                    )
                    # ACT's HWDGE ring: outputs don't queue behind input DMAs
                    nc.scalar.dma_start(
                        qr_d[nb * P:(nb + 1) * P, mt * M_TILE:(mt + 1) * M_TILE],
                        q_t[:],
                    )
    nc.compile()
    return nc


def _get_nc():
    global _NC
    if _NC is None:
        _NC = _build_program()
    return _NC


def kernel(x: np.ndarray, weight: np.ndarray):
    global LAST_RESULTS
    x = np.asarray(x)
    weight = np.asarray(weight)
    assert x.shape == (B, S, K) and weight.shape == (N_TOT, K)

    # Host prep: transpose; bf16 / int8 casts are exact for |v| <= 128
    xt = x.reshape(M, K).T  # [K, M] view
    rows_b = np.concatenate([np.arange(kb * P, (kb + 1) * P) for kb in BF16_KBS])
    rows_8 = np.concatenate([np.arange(kb * P, (kb + 1) * P) for kb in I8_KBS])
    xtb = np.ascontiguousarray(xt[rows_b].astype(np.float32).astype(BF16))
    xt8 = np.ascontiguousarray(xt[rows_8].astype(np.int8))
    in_maps = []
    for c in range(N_CORES):
        wt_c = weight[c * N_CORE:(c + 1) * N_CORE, :].T.astype(np.float32).astype(BF16)
        in_maps.append({"xtb": xtb, "xt8": xt8, "wt": np.ascontiguousarray(wt_c)})

    nc = _get_nc()
    res = bass_utils.run_bass_kernel_spmd(nc, in_maps, core_ids=list(range(N_CORES)))
    LAST_RESULTS = res

    qr = np.concatenate([r_["qr"] for r_ in res.results], axis=0)  # [N_TOT, M] int32
    qr = np.ascontiguousarray(qr.T)  # [M, N_TOT]
    q = (qr >> 16).astype(np.int64).reshape(B, S, N_TOT)
    r = (qr & 0xFFFF).astype(np.int64).reshape(B, S, N_TOT)
    return (q, r)
